# revision 5
# baseline (speedup 1.0000x reference)
"""Trainium2 Bass kernel for MixGRU: y = ((GRU_last(x @ Wmix.T)) @ Whead.T + bhead) @ Wmix.

Data-parallel over batch across 8 NeuronCores (32 batch elements per core).
All recurrent state kept transposed ([HID, B] tiles) so the sequential GRU
scan runs on cheap 96-partition ops. Input-side gate projections are
precomputed (fp32) in a pipelined fashion and injected into the scan's PSUM
banks via an identity-matmul (issued one step ahead, start=True); the
hidden-state matmuls run in bf16 (fp32 PSUM accumulate) with b_hh_n folded
in through hi/lo-split ones-rows of the state tile.
"""

import numpy as np

import concourse.bass as bass
import concourse.mybir as mybir
from concourse import bacc, tile
from concourse.bass_utils import run_bass_kernel_spmd

F32 = mybir.dt.float32
F16 = mybir.dt.float16
AFT = mybir.ActivationFunctionType
OP = mybir.AluOpType

B, T, D = 256, 512, 512
MIX, HID = 32, 96
NCORES = 8
BS = B // NCORES          # 32 batch per core
BLK = 16                  # scan steps per pipeline block
COLS = BLK * BS           # 512 columns per block
KH = HID + 2              # state rows + two ones-rows (bias hi/lo)

TRACE = False
LAST_EXEC_NS = None
_CACHE = {}


def build(t_total=T):
    nblk = t_total // BLK
    nc = bacc.Bacc("TRN2", target_bir_lowering=False, debug=False)

    xT = nc.dram_tensor("xT", [D, t_total * BS], F32, kind="ExternalInput")
    WzT = nc.dram_tensor("WzT", [128, 4, MIX], F32, kind="ExternalInput")
    Wih = nc.dram_tensor("Wih", [MIX + 1, 3 * HID], F32, kind="ExternalInput")
    # bf16 stationaries for the scan: gates r, u, -u, n ([KH, 4*HID]);
    # rows HID:KH are zeros except the n-gate, which carries b_hh_n hi/lo.
    Whh = nc.dram_tensor("Whh", [KH, 4 * HID], F16, kind="ExternalInput")
    I96 = nc.dram_tensor("I96", [HID, HID], F16, kind="ExternalInput")
    WheadT = nc.dram_tensor("WheadT", [HID, MIX], F32, kind="ExternalInput")
    bhead = nc.dram_tensor("bhead", [MIX, 1], F32, kind="ExternalInput")
    Wmix = nc.dram_tensor("Wmix", [MIX, D], F32, kind="ExternalInput")
    yT = nc.dram_tensor("yT", [D, BS], F32, kind="ExternalOutput")

    with tile.TileContext(nc) as tc:
        with (
            tc.tile_pool(name="wts", bufs=1) as wts,
            tc.tile_pool(name="xp", bufs=9) as xp,
            tc.tile_pool(name="zp", bufs=2) as zp,
            tc.tile_pool(name="gbp", bufs=3) as gbp,
            tc.tile_pool(name="gnp", bufs=3) as gnp,
            tc.tile_pool(name="hp", bufs=3) as hp,
            tc.tile_pool(name="gate", bufs=3) as gate,
            tc.tile_pool(name="outp", bufs=2) as outp,
            tc.tile_pool(name="zps", bufs=2, space="PSUM") as zps,
            tc.tile_pool(name="gxps", bufs=2, space="PSUM") as gxps,
            tc.tile_pool(name="ps1", bufs=2, space="PSUM") as ps1p,
            tc.tile_pool(name="ps2", bufs=2, space="PSUM") as ps2p,
        ):
            # ---- persistent weights in SBUF ----
            wz = wts.tile([128, 4, MIX], F32, tag="wz")
            nc.sync.dma_start(wz[:], WzT[:])
            wih = wts.tile([MIX + 1, 3 * HID], F32, tag="wih")
            nc.sync.dma_start(wih[:], Wih[:])
            whh = wts.tile([KH, 4 * HID], F16, tag="whh")
            nc.sync.dma_start(whh[:], Whh[:])
            i96 = wts.tile([HID, HID], F16, tag="i96")
            nc.sync.dma_start(i96[:], I96[:])
            whd = wts.tile([HID, MIX], F32, tag="whd")
            nc.sync.dma_start(whd[:], WheadT[:])
            bhd = wts.tile([MIX, 1], F32, tag="bhd")
            nc.sync.dma_start(bhd[:], bhead[:])
            wmx = wts.tile([MIX, D], F32, tag="wmx")
            nc.sync.dma_start(wmx[:], Wmix[:])

            # ---- ACT table warmup (sigmoid/tanh share one table set) ----
            scr = gate.tile([HID, BS], F32, tag="scr")
            nc.gpsimd.memset(scr[:], 0.0)
            nc.scalar.activation(scr[:], scr[:], AFT.Sigmoid)
            nc.scalar.activation(scr[:], scr[:], AFT.Tanh)

            # ---- initial hidden state ----
            h = hp.tile([KH, BS], F16)
            nc.gpsimd.memset(h[0:HID, :], 0.0)
            nc.gpsimd.memset(h[HID:KH, :], 1.0)

            def dma_block(j):
                xts = []
                for k in range(4):
                    xt = xp.tile([128, COLS], F32)
                    nc.sync.dma_start(
                        xt[:], xT[k * 128:(k + 1) * 128, j * COLS:(j + 1) * COLS]
                    )
                    xts.append(xt)
                return xts

            def make_chunks(j, xts):
                """Precompute block j: returns (gb, gn, [chunk closures]).

                gb[:, i, :] holds bf16 (gxb_r | gxb_u | -gxb_u) for step i;
                gn holds fp32 gx_n (t-major, 32 batch cols per step)."""
                ztile = zp.tile([MIX + 1, COLS], F32)
                zpsum = zps.tile([MIX, COLS], F32)
                gb = gbp.tile([HID, BLK, 3 * BS], F16)
                gn = gnp.tile([HID, COLS], F32)

                def c0():
                    for k in range(2):
                        nc.tensor.matmul(
                            zpsum[:], wz[:, k, :], xts[k][:],
                            start=(k == 0), stop=False,
                        )

                def c1():
                    for k in range(2, 4):
                        nc.tensor.matmul(
                            zpsum[:], wz[:, k, :], xts[k][:],
                            start=False, stop=(k == 3),
                        )
                    nc.vector.tensor_copy(ztile[0:MIX, :], zpsum[:])
                    nc.gpsimd.memset(ztile[MIX:MIX + 1, :], 1.0)

                def c2():
                    for gi in range(2):  # r, u gates
                        gps = gxps.tile([HID, COLS], F32)
                        nc.tensor.matmul(
                            gps[:], wih[:, gi * HID:(gi + 1) * HID], ztile[:],
                            start=True, stop=True,
                        )
                        nc.vector.tensor_copy(
                            gb[:, :, gi * BS:(gi + 1) * BS],
                            gps[:].rearrange("p (t b) -> p t b", b=BS),
                        )
                        if gi == 1:  # negated copy for the (1-u) column
                            nc.vector.tensor_scalar(
                                gb[:, :, 2 * BS:3 * BS],
                                gps[:].rearrange("p (t b) -> p t b", b=BS),
                                -1.0, None, op0=OP.mult,
                            )

                def c3():
                    gps = gxps.tile([HID, COLS], F32)
                    nc.tensor.matmul(
                        gps[:], wih[:, 2 * HID:3 * HID], ztile[:],
                        start=True, stop=True,
                    )
                    nc.vector.tensor_copy(gn[:], gps[:])

                return gb, gn, [c0, c1, c2, c3]

            def imm(gb, i):
                """Inject precomputed gate inputs for step i into a fresh
                PSUM bank (start=True) — issued one step ahead."""
                ps1 = ps1p.tile([HID, 3 * BS], F32, tag="ps1")
                nc.tensor.matmul(ps1[:], i96[:], gb[:, i, :],
                                 start=True, stop=False)
                return ps1

            def scan_step(h, ps1, gn, i):
                nc.tensor.matmul(ps1[:, 0:BS], whh[:, 0:HID], h[:],
                                 start=False, stop=False)
                nc.tensor.matmul(ps1[:, BS:2 * BS], whh[:, HID:2 * HID], h[:],
                                 start=False, stop=False)
                nc.tensor.matmul(ps1[:, 2 * BS:3 * BS], whh[:, 2 * HID:3 * HID],
                                 h[:], start=False, stop=True)
                ps2 = ps2p.tile([HID, 2 * BS], F32, tag="ps2")
                nc.tensor.matmul(ps2[:, 0:BS], whh[:, 3 * HID:4 * HID], h[:],
                                 start=True, stop=True)

                r = gate.tile([HID, BS], F32, tag="r")
                nc.scalar.activation(r[:], ps1[:, 0:BS], AFT.Sigmoid)
                uu = gate.tile([HID, 2 * BS], F32, tag="uu")
                nc.scalar.activation(uu[:], ps1[:, BS:3 * BS], AFT.Sigmoid)

                tn = gate.tile([HID, BS], F32, tag="tn")
                nc.vector.tensor_tensor(tn[:], ps2[:, 0:BS], r[:], op=OP.mult)
                nc.vector.tensor_tensor(
                    ps2[:, BS:2 * BS], tn[:], gn[:, i * BS:(i + 1) * BS],
                    op=OP.add,
                )
                nn = gate.tile([HID, BS], F32, tag="nn")
                nc.scalar.activation(nn[:], ps2[:, BS:2 * BS], AFT.Tanh)

                uh = gate.tile([HID, BS], F32, tag="uh")
                nc.vector.tensor_tensor(uh[:], uu[:, 0:BS], h[0:HID, :],
                                        op=OP.mult)
                h2 = hp.tile([KH, BS], F16)
                nc.gpsimd.memset(h2[HID:KH, :], 1.0)
                nc.vector.tensor_tensor(h2[0:HID, :], uu[:, BS:2 * BS], nn[:],
                                        op=OP.mult)
                nc.vector.tensor_tensor(h2[0:HID, :], h2[0:HID, :], uh[:],
                                        op=OP.add)
                return h2

            # ---- pipelined precompute + scan ----
            xts0 = dma_block(0)
            xts1 = dma_block(1) if nblk > 1 else None
            gb, gn, chunks = make_chunks(0, xts0)
            for c in chunks:
                c()
            nxt = None
            if nblk > 1:
                nxt = make_chunks(1, xts1)

            ps1 = imm(gb, 0)
            for j in range(nblk):
                xts = dma_block(j + 2) if j + 2 < nblk else None
                cur_gb, cur_gn = gb, gn
                pend = nxt[2] if nxt is not None else []
                nxt_tiles = (nxt[0], nxt[1]) if nxt is not None else None
                nxt = make_chunks(j + 2, xts) if xts is not None else None
                for i in range(BLK):
                    h2 = scan_step(h, ps1, cur_gn, i)
                    # inject next step's gate inputs while this chain runs
                    last = (j == nblk - 1) and (i == BLK - 1)
                    if not last:
                        if i == BLK - 1:
                            ps1 = imm(nxt_tiles[0], 0)
                        else:
                            ps1 = imm(cur_gb, i + 1)
                    h = h2
                    if i % 2 == 1 and (i // 2) < len(pend):
                        pend[i // 2]()
                if nxt_tiles is not None:
                    gb, gn = nxt_tiles

            # ---- head: z_next = Whead @ h + bhead ; y.T = Wmix.T @ z_next ----
            hf = gate.tile([HID, BS], F32, tag="hf")
            nc.vector.tensor_copy(hf[:], h[0:HID, :])
            znps = ps1p.tile([MIX, BS], F32, tag="ps1")
            nc.tensor.matmul(znps[:], whd[:], hf[:], start=True, stop=True)
            zn = gate.tile([MIX, BS], F32, tag="zn")
            nc.vector.tensor_scalar(zn[:], znps[:], bhd[:], None, op0=OP.add)
            for k in range(4):
                yps = ps2p.tile([128, BS], F32, tag="ps2")
                nc.tensor.matmul(yps[:], wmx[:, k * 128:(k + 1) * 128], zn[:],
                                 start=True, stop=True)
                yt = outp.tile([128, BS], F32)
                nc.vector.tensor_copy(yt[:], yps[:])
                nc.sync.dma_start(yT[k * 128:(k + 1) * 128, :], yt[:])

    nc.compile()
    return nc


def _f16(a):
    return np.asarray(a, np.float32).astype(np.float16)


def prep_weights(W_mix, W_ih, W_hh, b_ih, b_hh, W_head, b_head):
    W_mix = np.asarray(W_mix, np.float32)
    W_ih = np.asarray(W_ih, np.float32)
    W_hh = np.asarray(W_hh, np.float32)
    b_ih = np.asarray(b_ih, np.float32)
    b_hh = np.asarray(b_hh, np.float32)
    W_head = np.asarray(W_head, np.float32)
    b_head = np.asarray(b_head, np.float32)

    # WzT[p, k, m] = W_mix[m, 128k + p]
    WzT = np.ascontiguousarray(
        W_mix.T.reshape(4, 128, MIX).transpose(1, 0, 2)
    )
    # Wih_hat: [MIX+1, 3H]; per gate columns = [W_ih_g.T ; fused bias]
    gates_b = [
        b_ih[0:HID] + b_hh[0:HID],
        b_ih[HID:2 * HID] + b_hh[HID:2 * HID],
        b_ih[2 * HID:3 * HID],
    ]
    Wih_hat = np.zeros((MIX + 1, 3 * HID), np.float32)
    for g in range(3):
        Wih_hat[0:MIX, g * HID:(g + 1) * HID] = W_ih[g * HID:(g + 1) * HID].T
        Wih_hat[MIX, g * HID:(g + 1) * HID] = gates_b[g]

    # bf16 scan stationaries [KH, 4H]: r, u, -u, n; n carries b_hh_n hi/lo.
    Whh_hat = np.zeros((KH, 4 * HID), np.float32)
    Wr, Wu, Wn = (W_hh[g * HID:(g + 1) * HID] for g in range(3))
    Whh_hat[0:HID, 0:HID] = Wr.T
    Whh_hat[0:HID, HID:2 * HID] = Wu.T
    Whh_hat[0:HID, 2 * HID:3 * HID] = -Wu.T
    Whh_hat[0:HID, 3 * HID:4 * HID] = Wn.T
    bn = b_hh[2 * HID:3 * HID]
    bn_hi = bn.astype(np.float16).astype(np.float32)
    Whh_hat[HID, 3 * HID:4 * HID] = bn_hi
    Whh_hat[HID + 1, 3 * HID:4 * HID] = bn - bn_hi
    return {
        "WzT": WzT,
        "Wih": Wih_hat,
        "Whh": _f16(Whh_hat),
        "I96": _f16(np.eye(HID, dtype=np.float32)),
        "WheadT": np.ascontiguousarray(W_head.T),
        "bhead": np.ascontiguousarray(b_head[:, None]),
        "Wmix": W_mix,
    }


def kernel(x, W_mix, W_ih, W_hh, b_ih, b_hh, W_head, b_head):
    global LAST_EXEC_NS
    if "nc" not in _CACHE:
        _CACHE["nc"] = build(T)
    nc = _CACHE["nc"]

    wmap = prep_weights(W_mix, W_ih, W_hh, b_ih, b_hh, W_head, b_head)
    x = np.asarray(x, np.float32)
    in_maps = []
    for c in range(NCORES):
        xc = x[c * BS:(c + 1) * BS]                       # [BS, T, D]
        xTc = np.ascontiguousarray(xc.transpose(2, 1, 0)).reshape(D, T * BS)
        in_maps.append({"xT": xTc, **wmap})

    res = run_bass_kernel_spmd(
        nc, in_maps, core_ids=list(range(NCORES)), trace=TRACE
    )
    LAST_EXEC_NS = res.exec_time_ns
    y = np.empty((B, D), np.float32)
    for c in range(NCORES):
        y[c * BS:(c + 1) * BS] = res.results[c]["yT"].T
    return y
